# revision 35
# baseline (speedup 1.0000x reference)
"""Trainium2 Bass kernel for nn_CINComp: out[b,o,d] = sum_{i,j} W[o,i*64+j]*feature[b,i,d]*base[b,j,d] + bias[o].

Sharding: data-parallel over batch B=1024 across 8 cores (128 batches/core).
Per-core algorithm (all shapes hardcoded):
  - contraction dim ij = 4096 split into 32 K-chunks of 128 = (2 i-rows x 64 j)
  - G is stored transposed+duplicated: gt2[p=(dup,j), (b,d)] so each K-chunk's
    G-factor is a static 128-partition tile slice
  - per chunk, a K=8 selector matmul on the PE broadcasts the chunk's 2 F-rows
    across the partition halves into PSUM (fp32r, 1 cyc/row)
  - DVE multiplies gt2-slice * F-broadcast -> P chunk (the outer-product block)
  - PE contracts W^T-chunk (fp32r) @ P accumulating out[o,(b,d)] in PSUM
  - ScalarE adds bias during PSUM->SBUF copy, DMA out
"""

import numpy as np

import concourse.bass as bass
import concourse.mybir as mybir
import concourse.tile as tile
from concourse.bass import ts
from concourse.bass_utils import run_bass_kernel_spmd

B, HK, H0, D, O = 1024, 64, 64, 32, 128
NCORES = 8
BLOC = B // NCORES          # 128 batches per core
GROUPS = 8                  # batch groups per core
GB = BLOC // GROUPS         # 16 batches per group
N = GB * D                  # 512 = matmul free dim per group
NCHUNK = 32                 # K chunks of 128 over ij=4096
F32 = mybir.dt.float32
F32R = mybir.dt.float32r

_CACHE = {}


def _sel_const() -> np.ndarray:
    # sel[k, c, m] = 1 iff k == 2c + (m // 64): K=128 selector per chunk c that
    # broadcasts F-row 2c+delta (at partition 2c+delta of the padded F tile)
    # to partition half delta. K=128 keeps the PE's fused weight load on the
    # fast path (FWL requires 128 weight rows).
    sel = np.zeros((128, NCHUNK, 128), np.float32)
    for c in range(NCHUNK):
        for m in range(128):
            sel[2 * c + (m // 64), c, m] = 1.0
    return sel.reshape(128, NCHUNK * 128)


def _strip_self_waits(nc: bass.Bass) -> None:
    """Transitively-minimal semaphore waits.

    Tile's add_semaphores pass is per-proc minimal but not transitively
    minimal, and several instruction structs (fused LDWEIGHTS+MATMUL, TS, TT,
    DMA descriptors) accept only ONE wait.  This pass computes completion
    vector clocks (sem id -> value) for every instruction, exploiting
    in-order FIFO execution (per engine; per DMA queue = per queue sem), and
    drops any wait already implied by the instruction's FIFO predecessor or
    by its remaining waits.
    """
    UPD = ("sem-inc", "sem-add-imm")
    insts = [i for bb in nc.m.functions[0].blocks for i in bb.instructions]

    # Sems driven by anything other than monotonic inc/add (e.g. the barrier
    # protocol's sem-sub) must keep their waits untouched.
    bad_sems = set()
    for i in insts:
        si = getattr(i, "sync_info", None)
        if si is None:
            continue
        for u in si.on_update:
            if u.sync_type != "semaphore" or u.update_mode not in UPD:
                bad_sems.add(u.id)

    def fifo_of(i):
        si = i.sync_info
        eng = str(getattr(i, "engine", None))
        if type(i).__name__ == "InstDMACopy" and si is not None:
            for u in si.on_update:
                if u.sync_type == "semaphore" and u.update_mode in UPD:
                    return ("q", u.id)
        return ("e", eng)

    # map sem threshold -> updater index, in program order (valid because all
    # updaters of one sem share a FIFO)
    cum: dict = {}
    event: dict = {}  # (sem, cum_value_after_update) -> inst index
    fifo_pred: dict = {}
    last_in_fifo: dict = {}
    metas = []
    for idx, i in enumerate(insts):
        si = getattr(i, "sync_info", None)
        f = fifo_of(i)
        fifo_pred[idx] = last_in_fifo.get(f)
        last_in_fifo[f] = idx
        ups = []
        if si is not None:
            for u in si.on_update:
                if u.sync_type == "semaphore" and u.update_mode in UPD:
                    cum[u.id] = cum.get(u.id, 0) + u.update_value
                    event[(u.id, cum[u.id])] = idx
                    ups.append((u.id, cum[u.id]))
        metas.append((si, ups))

    # resolve a wait (sem, k) to the earliest event index with cum >= k
    def resolve(sem, k):
        v = k
        while (sem, v) not in event:
            v += 1
            if v > cum.get(sem, 0):
                return None
        return event[(sem, v)]

    cvc: list = [None] * len(insts)

    def get_cvc(idx):
        if cvc[idx] is not None:
            return cvc[idx]
        # iterative DFS to avoid deep recursion
        stack = [idx]
        while stack:
            j = stack[-1]
            if cvc[j] is not None:
                stack.pop()
                continue
            si, ups = metas[j]
            deps = []
            p = fifo_pred[j]
            if p is not None:
                deps.append(p)
            if si is not None:
                for w in si.on_wait:
                    if (
                        w.sync_type == "semaphore"
                        and w.wait_mode == "sem-ge-imm"
                        and w.id not in bad_sems
                    ):
                        e = resolve(w.id, w.wait_value)
                        if e is not None and e != j:
                            deps.append(e)
            pending = [d for d in deps if cvc[d] is None]
            if pending:
                stack.extend(pending)
                continue
            stack.pop()
            vc: dict = {}
            for d in deps:
                for s, v in cvc[d].items():
                    if vc.get(s, 0) < v:
                        vc[s] = v
            if si is not None:
                for w in si.on_wait:
                    if (
                        w.sync_type == "semaphore"
                        and w.wait_mode == "sem-ge-imm"
                        and w.id not in bad_sems
                    ):
                        if vc.get(w.id, 0) < w.wait_value:
                            vc[w.id] = w.wait_value
            for s, v in ups:
                if vc.get(s, 0) < v:
                    vc[s] = v
            cvc[j] = vc
        return cvc[idx]

    for idx, i in enumerate(insts):
        si, _ups = metas[idx]
        if si is None or not si.on_wait:
            continue
        base: dict = {}
        p = fifo_pred[idx]
        if p is not None:
            base = dict(get_cvc(p))
        sem_waits = [
            w
            for w in si.on_wait
            if w.sync_type == "semaphore"
            and w.wait_mode == "sem-ge-imm"
            and w.id not in bad_sems
        ]
        other = [w for w in si.on_wait if w not in sem_waits]
        # strongest-first so one kept wait can subsume the rest
        def strength(w):
            e = resolve(w.id, w.wait_value)
            return len(get_cvc(e)) if e is not None else 0

        sem_waits.sort(key=strength, reverse=True)

        def wait_cvc(w):
            e = resolve(w.id, w.wait_value)
            vc = dict(get_cvc(e)) if e is not None else {}
            if vc.get(w.id, 0) < w.wait_value:
                vc[w.id] = w.wait_value
            return vc

        kept = sem_waits[:]
        changed = True
        while changed:
            changed = False
            for w in kept:
                cover = dict(base)
                for w2 in kept:
                    if w2 is w:
                        continue
                    for s, v in wait_cvc(w2).items():
                        if cover.get(s, 0) < v:
                            cover[s] = v
                if cover.get(w.id, 0) >= w.wait_value:
                    kept.remove(w)
                    changed = True
                    break
        if len(kept) + len(other) != len(si.on_wait):
            si.on_wait = other + kept


def _build_nc(strip: bool = True) -> bass.Bass:
    nc = bass.Bass()
    wt = nc.dram_tensor("wt", [128, NCHUNK * 128], F32R, kind="ExternalInput")
    gt2 = nc.dram_tensor("gt2", [128, BLOC * D], F32, kind="ExternalInput")
    ftp = nc.dram_tensor("ftp", [128, BLOC * D], F32R, kind="ExternalInput")
    bias = nc.dram_tensor("bias", [128, 1], F32, kind="ExternalInput")
    out = nc.dram_tensor("out", [128, BLOC * D], F32, kind="ExternalOutput")
    sel_d = nc.dram_tensor("sel", [128, NCHUNK * 128], F32R, kind="ExternalInput")

    PAIRS = NCHUNK // 2  # chunks processed two-at-a-time

    with tile.TileContext(nc) as tc:
        with (
            tc.tile_pool(name="resident", bufs=1) as res,
            tc.tile_pool(name="p", bufs=3) as ppool,
            tc.tile_pool(name="osb", bufs=8) as opool,
            tc.tile_pool(name="tiny", bufs=2) as tpool,
            tc.tile_pool(name="fbc", bufs=3, space="PSUM") as fpool,
            tc.tile_pool(name="acc", bufs=2, space="PSUM") as apool,
        ):
            gt2_sb = res.tile([128, BLOC * D], F32)
            nc.sync.dma_start(out=gt2_sb[:], in_=gt2[:])
            ft_sb = res.tile([128, BLOC * D], F32R)
            nc.sync.dma_start(out=ft_sb[:], in_=ftp[:])
            sel_sb = res.tile([128, NCHUNK * 128], F32R)
            nc.sync.dma_start(out=sel_sb[:], in_=sel_d[:])
            wt_sb = res.tile([128, NCHUNK * 128], F32R)
            nc.sync.dma_start(out=wt_sb[:], in_=wt[:])
            bias_sb = res.tile([128, 1], F32)
            nc.sync.dma_start(out=bias_sb[:], in_=bias[:])

            # Touch each resident tile with a 1-element self-copy. The write
            # makes every later reader depend on the touching engine instead
            # of the DMA queue, consolidating DMA waits onto one sem (the
            # instruction structs accept only one embedded wait), and the
            # RAW dep keeps the touch ordered before the hot loop.
            nc.scalar.copy(ft_sb[0:1, 0:1], ft_sb[0:1, 0:1])
            nc.scalar.copy(sel_sb[0:1, 0:1], sel_sb[0:1, 0:1])
            nc.scalar.copy(gt2_sb[0:1, 0:1], gt2_sb[0:1, 0:1])
            nc.scalar.copy(bias_sb[0:1, 0:1], bias_sb[0:1, 0:1])
            nc.vector.tensor_copy(wt_sb[0:1, 0:1], wt_sb[0:1, 0:1])
            tiny = tpool.tile([128, 1], F32, tag="tiny")
            nc.vector.tensor_copy(tiny[:], bias_sb[:, 0:1])

            # Per-pair lane: A = ACT-stage + DVE-mult, B = DVE direct from
            # PSUM, C = ACT-stage + GPSIMD-mult. Spreads the Hadamard work
            # across all three elementwise-capable engines.
            LANES = ["A", "C", "A", "B", "A", "C", "A", "B",
                     "A", "C", "A", "B", "A", "C", "A", "C"]
            prev_osb = None
            for g in range(GROUPS):
                if prev_osb is not None:
                    # advance GPSIMD's PE clock past group g-1's matmuls so
                    # its pg-slot reuse waits are already satisfied
                    gsync = tpool.tile([128, 1], F32, tag="gsync")
                    nc.gpsimd.tensor_copy(gsync[:], prev_osb[:, 0:1])
                acc = apool.tile([128, N], F32, tag="acc")
                for k in range(PAIRS):
                    lane = LANES[k]
                    c0, c1 = 2 * k, 2 * k + 1
                    # two K=128 broadcast matmuls -> one 2-bank PSUM tile
                    fbc = fpool.tile([128, 2 * N], F32, tag="fbc")
                    nc.tensor.matmul(fbc[:, 0:N], sel_sb[:, ts(c0, 128)],
                                     ft_sb[:, ts(g, N)], start=True, stop=True)
                    nc.tensor.matmul(fbc[:, N:2 * N], sel_sb[:, ts(c1, 128)],
                                     ft_sb[:, ts(g, N)], start=True, stop=True)
                    gview = gt2_sb[:, ts(g, N)][:, None, :].to_broadcast(
                        (128, 2, N))
                    if lane == "B":
                        p = ppool.tile([128, 2, N], F32R, tag="pd", bufs=2)
                        nc.vector.tensor_mul(
                            p[:], gview,
                            fbc[:].rearrange("a (b n) -> a b n", b=2))
                    else:
                        tag = "pa" if lane == "A" else "pg"
                        p = ppool.tile([128, 2, N], F32R, tag=tag,
                                       bufs=8 if lane == "A" else 5)
                        nc.scalar.copy(
                            p[:], fbc[:].rearrange("a (b n) -> a b n", b=2))
                        eng = nc.vector if lane == "A" else nc.gpsimd
                        eng.tensor_mul(p[:], gview, p[:])
                    nc.tensor.matmul(acc[:], wt_sb[:, ts(c0, 128)], p[:, 0, :],
                                     start=(k == 0), stop=False)
                    nc.tensor.matmul(acc[:], wt_sb[:, ts(c1, 128)], p[:, 1, :],
                                     start=False, stop=(k == PAIRS - 1))

                osb = opool.tile([128, N], F32, tag="osb")
                nc.vector.tensor_scalar(osb[:], acc[:], bias_sb[:, 0:1],
                                        None, mybir.AluOpType.add)
                nc.sync.dma_start(out=out[:, ts(g, N)], in_=osb[:])
                # WAR consumer: pulls the out-DMA's queue-sem onto the DVE
                # clock so the kernel-tail drain needs only one wait.
                nc.vector.tensor_copy(osb[0:1, 0:1], tiny[0:1, 0:1])
                prev_osb = osb

    if strip:
        _strip_self_waits(nc)
    return nc


def _get_nc() -> bass.Bass:
    if "nc" not in _CACHE:
        _CACHE["nc"] = _build_nc()
    return _CACHE["nc"]


def _prep_core_inputs(feature, base, W, b, ci):
    bsl = slice(ci * BLOC, (ci + 1) * BLOC)
    F = np.ascontiguousarray(feature[bsl], np.float32)  # (128, 64, 32)
    G = np.ascontiguousarray(base[bsl], np.float32)     # (128, 64, 32)

    Gt = np.transpose(G, (1, 0, 2))                      # (j, b, d)
    gt2 = np.concatenate([Gt, Gt], 0).reshape(128, BLOC * D)

    # ftp: F transposed to (i, b, d), padded with zeros to 128 partitions
    Ft = np.transpose(F, (1, 0, 2)).reshape(HK, BLOC * D)
    ftp = np.concatenate([Ft, np.zeros_like(Ft)], 0)

    # wt[p, c, o] = W[o, 128c + p]
    wt = np.transpose(W.reshape(O, NCHUNK, 128), (2, 1, 0)).reshape(128, NCHUNK * 128)

    return {
        "wt": np.ascontiguousarray(wt, np.float32),
        "gt2": np.ascontiguousarray(gt2, np.float32),
        "ftp": np.ascontiguousarray(ftp, np.float32),
        "bias": np.ascontiguousarray(b, np.float32).reshape(128, 1),
        "sel": _sel_const(),
    }


def run(feature, base, W, b, **spmd_kwargs):
    nc = _get_nc()
    in_maps = [_prep_core_inputs(feature, base, W, b, ci) for ci in range(NCORES)]
    res = run_bass_kernel_spmd(nc, in_maps, list(range(NCORES)), **spmd_kwargs)
    outs = []
    for ci in range(NCORES):
        o = res.results[ci]["out"].reshape(O, BLOC, D)
        outs.append(np.transpose(o, (1, 0, 2)))
    full = np.concatenate(outs, 0)
    return full, res


def kernel(feature, base, W, b):
    full, _ = run(feature, base, W, b)
    return full


# revision 39
# speedup vs baseline: 1.1985x; 1.1985x over previous
"""Trainium2 Bass kernel for nn_CINComp: out[b,o,d] = sum_{i,j} W[o,i*64+j]*feature[b,i,d]*base[b,j,d] + bias[o].

Sharding: data-parallel over batch B=1024 across 8 cores (128 batches/core).
Per-core algorithm (all shapes hardcoded):
  - contraction dim ij = 4096 split into 32 K-chunks of 128 = (2 i-rows x 64 j)
  - G is stored transposed+duplicated: gt2[p=(dup,j), (b,d)] so each K-chunk's
    G-factor is a static 128-partition tile slice
  - per chunk, a K=8 selector matmul on the PE broadcasts the chunk's 2 F-rows
    across the partition halves into PSUM (fp32r, 1 cyc/row)
  - DVE multiplies gt2-slice * F-broadcast -> P chunk (the outer-product block)
  - PE contracts W^T-chunk (fp32r) @ P accumulating out[o,(b,d)] in PSUM
  - ScalarE adds bias during PSUM->SBUF copy, DMA out
"""

import numpy as np

import concourse.bass as bass
import concourse.mybir as mybir
import concourse.tile as tile
from concourse.bass import ts
from concourse.bass_utils import run_bass_kernel_spmd

B, HK, H0, D, O = 1024, 64, 64, 32, 128
NCORES = 8
BLOC = B // NCORES          # 128 batches per core
GROUPS = 8                  # batch groups per core
GB = BLOC // GROUPS         # 16 batches per group
N = GB * D                  # 512 = matmul free dim per group
NCHUNK = 32                 # K chunks of 128 over ij=4096
F32 = mybir.dt.float32
F32R = mybir.dt.float32r

_CACHE = {}


def _sel_const() -> np.ndarray:
    # sel[k, c, m] = 1 iff k == 2c + (m // 64): K=128 selector per chunk c that
    # broadcasts F-row 2c+delta (at partition 2c+delta of the padded F tile)
    # to partition half delta. K=128 keeps the PE's fused weight load on the
    # fast path (FWL requires 128 weight rows).
    sel = np.zeros((128, NCHUNK, 128), np.float32)
    for c in range(NCHUNK):
        for m in range(128):
            sel[2 * c + (m // 64), c, m] = 1.0
    return sel.reshape(128, NCHUNK * 128)


def _strip_self_waits(nc: bass.Bass) -> None:
    """Transitively-minimal semaphore waits.

    Tile's add_semaphores pass is per-proc minimal but not transitively
    minimal, and several instruction structs (fused LDWEIGHTS+MATMUL, TS, TT,
    DMA descriptors) accept only ONE wait.  This pass computes completion
    vector clocks (sem id -> value) for every instruction, exploiting
    in-order FIFO execution (per engine; per DMA queue = per queue sem), and
    drops any wait already implied by the instruction's FIFO predecessor or
    by its remaining waits.
    """
    UPD = ("sem-inc", "sem-add-imm")
    insts = [i for bb in nc.m.functions[0].blocks for i in bb.instructions]

    # Sems driven by anything other than monotonic inc/add (e.g. the barrier
    # protocol's sem-sub) must keep their waits untouched.
    bad_sems = set()
    for i in insts:
        si = getattr(i, "sync_info", None)
        if si is None:
            continue
        for u in si.on_update:
            if u.sync_type != "semaphore" or u.update_mode not in UPD:
                bad_sems.add(u.id)

    def fifo_of(i):
        si = i.sync_info
        eng = str(getattr(i, "engine", None))
        if type(i).__name__ == "InstDMACopy" and si is not None:
            for u in si.on_update:
                if u.sync_type == "semaphore" and u.update_mode in UPD:
                    return ("q", u.id)
        return ("e", eng)

    # map sem threshold -> updater index, in program order (valid because all
    # updaters of one sem share a FIFO)
    cum: dict = {}
    event: dict = {}  # (sem, cum_value_after_update) -> inst index
    fifo_pred: dict = {}
    last_in_fifo: dict = {}
    metas = []
    for idx, i in enumerate(insts):
        si = getattr(i, "sync_info", None)
        f = fifo_of(i)
        fifo_pred[idx] = last_in_fifo.get(f)
        last_in_fifo[f] = idx
        ups = []
        if si is not None:
            for u in si.on_update:
                if u.sync_type == "semaphore" and u.update_mode in UPD:
                    cum[u.id] = cum.get(u.id, 0) + u.update_value
                    event[(u.id, cum[u.id])] = idx
                    ups.append((u.id, cum[u.id]))
        metas.append((si, ups))

    # resolve a wait (sem, k) to the earliest event index with cum >= k
    def resolve(sem, k):
        v = k
        while (sem, v) not in event:
            v += 1
            if v > cum.get(sem, 0):
                return None
        return event[(sem, v)]

    cvc: list = [None] * len(insts)

    def get_cvc(idx):
        if cvc[idx] is not None:
            return cvc[idx]
        # iterative DFS to avoid deep recursion
        stack = [idx]
        while stack:
            j = stack[-1]
            if cvc[j] is not None:
                stack.pop()
                continue
            si, ups = metas[j]
            deps = []
            p = fifo_pred[j]
            if p is not None:
                deps.append(p)
            if si is not None:
                for w in si.on_wait:
                    if (
                        w.sync_type == "semaphore"
                        and w.wait_mode == "sem-ge-imm"
                        and w.id not in bad_sems
                    ):
                        e = resolve(w.id, w.wait_value)
                        if e is not None and e != j:
                            deps.append(e)
            pending = [d for d in deps if cvc[d] is None]
            if pending:
                stack.extend(pending)
                continue
            stack.pop()
            vc: dict = {}
            for d in deps:
                for s, v in cvc[d].items():
                    if vc.get(s, 0) < v:
                        vc[s] = v
            if si is not None:
                for w in si.on_wait:
                    if (
                        w.sync_type == "semaphore"
                        and w.wait_mode == "sem-ge-imm"
                        and w.id not in bad_sems
                    ):
                        if vc.get(w.id, 0) < w.wait_value:
                            vc[w.id] = w.wait_value
            for s, v in ups:
                if vc.get(s, 0) < v:
                    vc[s] = v
            cvc[j] = vc
        return cvc[idx]

    for idx, i in enumerate(insts):
        si, _ups = metas[idx]
        if si is None or not si.on_wait:
            continue
        base: dict = {}
        p = fifo_pred[idx]
        if p is not None:
            base = dict(get_cvc(p))
        sem_waits = [
            w
            for w in si.on_wait
            if w.sync_type == "semaphore"
            and w.wait_mode == "sem-ge-imm"
            and w.id not in bad_sems
        ]
        other = [w for w in si.on_wait if w not in sem_waits]
        # strongest-first so one kept wait can subsume the rest
        def strength(w):
            e = resolve(w.id, w.wait_value)
            return len(get_cvc(e)) if e is not None else 0

        sem_waits.sort(key=strength, reverse=True)

        def wait_cvc(w):
            e = resolve(w.id, w.wait_value)
            vc = dict(get_cvc(e)) if e is not None else {}
            if vc.get(w.id, 0) < w.wait_value:
                vc[w.id] = w.wait_value
            return vc

        kept = sem_waits[:]
        changed = True
        while changed:
            changed = False
            for w in kept:
                cover = dict(base)
                for w2 in kept:
                    if w2 is w:
                        continue
                    for s, v in wait_cvc(w2).items():
                        if cover.get(s, 0) < v:
                            cover[s] = v
                if cover.get(w.id, 0) >= w.wait_value:
                    kept.remove(w)
                    changed = True
                    break
        if len(kept) + len(other) != len(si.on_wait):
            si.on_wait = other + kept


def _build_nc(strip: bool = True) -> bass.Bass:
    nc = bass.Bass()
    wt = nc.dram_tensor("wt", [128, NCHUNK * 128], F32R, kind="ExternalInput")
    gt2 = nc.dram_tensor("gt2", [128, BLOC * D], F32, kind="ExternalInput")
    ftp = nc.dram_tensor("ftp", [128, BLOC * D], F32R, kind="ExternalInput")
    bias = nc.dram_tensor("bias", [128, 1], F32, kind="ExternalInput")
    out = nc.dram_tensor("out", [128, BLOC * D], F32, kind="ExternalOutput")
    sel_d = nc.dram_tensor("sel", [128, NCHUNK * 128], F32R, kind="ExternalInput")

    PAIRS = NCHUNK // 2  # chunks processed two-at-a-time

    with tile.TileContext(nc) as tc:
        with (
            tc.tile_pool(name="resident", bufs=1) as res,
            tc.tile_pool(name="p", bufs=3) as ppool,
            tc.tile_pool(name="osb", bufs=8) as opool,
            tc.tile_pool(name="tiny", bufs=2) as tpool,
            tc.tile_pool(name="fbc", bufs=3, space="PSUM") as fpool,
            tc.tile_pool(name="acc", bufs=2, space="PSUM") as apool,
        ):
            gt2_sb = res.tile([128, BLOC * D], F32)
            ft_sb = res.tile([128, BLOC * D], F32R)
            sel_sb = res.tile([128, NCHUNK * 128], F32R)
            wt_sb = res.tile([128, NCHUNK * 128], F32R)
            bias_sb = res.tile([128, 1], F32)

            # Quarter-sliced resident loads, interleaved so the first pairs'
            # operands land early and compute overlaps the remaining input
            # DMA. Each piece is "touched" with a 1-element self-copy on its
            # consumer engine: later readers then depend on that engine's sem
            # instead of the DMA queue (instruction structs accept only one
            # embedded wait), and the RAW dep keeps the ordering.
            Q = BLOC * D // 4
            nc.sync.dma_start(out=bias_sb[:], in_=bias[:])
            for q in range(4):
                nc.sync.dma_start(out=gt2_sb[:, ts(q, Q)], in_=gt2[:, ts(q, Q)])
                nc.sync.dma_start(out=ft_sb[:, ts(q, Q)], in_=ftp[:, ts(q, Q)])
                nc.sync.dma_start(out=sel_sb[:, ts(q, Q)], in_=sel_d[:, ts(q, Q)])
                nc.sync.dma_start(out=wt_sb[:, ts(q, Q)], in_=wt[:, ts(q, Q)])
                for t in (ft_sb, sel_sb, gt2_sb, wt_sb):
                    nc.vector.tensor_copy(t[0:1, q * Q:q * Q + 1],
                                          t[0:1, q * Q:q * Q + 1])
            nc.vector.tensor_copy(bias_sb[0:1, 0:1], bias_sb[0:1, 0:1])
            tiny = tpool.tile([128, 1], F32, tag="tiny")
            nc.vector.tensor_copy(tiny[:], bias_sb[:, 0:1])

            # All pairs: DVE multiplies G in directly from the PSUM broadcast
            # tile (measured same DVE rate as SBUF-staged, and it frees ACT).
            LANES = ["B"] * PAIRS
            for g in range(GROUPS):
                acc = apool.tile([128, N], F32, tag="acc")
                for k in range(PAIRS):
                    lane = LANES[k]
                    c0, c1 = 2 * k, 2 * k + 1
                    # two K=128 broadcast matmuls -> one 2-bank PSUM tile
                    fbc = fpool.tile([128, 2 * N], F32, tag="fbc")
                    nc.tensor.matmul(fbc[:, 0:N], sel_sb[:, ts(c0, 128)],
                                     ft_sb[:, ts(g, N)], start=True, stop=True)
                    nc.tensor.matmul(fbc[:, N:2 * N], sel_sb[:, ts(c1, 128)],
                                     ft_sb[:, ts(g, N)], start=True, stop=True)
                    gview = gt2_sb[:, ts(g, N)][:, None, :].to_broadcast(
                        (128, 2, N))
                    p = ppool.tile([128, 2, N], F32R, tag="pd", bufs=3)
                    nc.vector.tensor_mul(
                        p[:], gview,
                        fbc[:].rearrange("a (b n) -> a b n", b=2))
                    nc.tensor.matmul(acc[:], wt_sb[:, ts(c0, 128)], p[:, 0, :],
                                     start=(k == 0), stop=False)
                    nc.tensor.matmul(acc[:], wt_sb[:, ts(c1, 128)], p[:, 1, :],
                                     start=False, stop=(k == PAIRS - 1))

                osb = opool.tile([128, N], F32, tag="osb")
                nc.vector.tensor_scalar(osb[:], acc[:], bias_sb[:, 0:1],
                                        None, mybir.AluOpType.add)
                nc.sync.dma_start(out=out[:, ts(g, N)], in_=osb[:])
                # WAR consumer: pulls the out-DMA's queue-sem onto the DVE
                # clock so the kernel-tail drain needs only one wait.
                nc.vector.tensor_copy(osb[0:1, 0:1], tiny[0:1, 0:1])
                prev_osb = osb

    if strip:
        _strip_self_waits(nc)
    return nc


def _get_nc() -> bass.Bass:
    if "nc" not in _CACHE:
        _CACHE["nc"] = _build_nc()
    return _CACHE["nc"]


def _prep_core_inputs(feature, base, W, b, ci):
    bsl = slice(ci * BLOC, (ci + 1) * BLOC)
    F = np.ascontiguousarray(feature[bsl], np.float32)  # (128, 64, 32)
    G = np.ascontiguousarray(base[bsl], np.float32)     # (128, 64, 32)

    Gt = np.transpose(G, (1, 0, 2))                      # (j, b, d)
    gt2 = np.concatenate([Gt, Gt], 0).reshape(128, BLOC * D)

    # ftp: F transposed to (i, b, d), padded with zeros to 128 partitions
    Ft = np.transpose(F, (1, 0, 2)).reshape(HK, BLOC * D)
    ftp = np.concatenate([Ft, np.zeros_like(Ft)], 0)

    # wt[p, c, o] = W[o, 128c + p]
    wt = np.transpose(W.reshape(O, NCHUNK, 128), (2, 1, 0)).reshape(128, NCHUNK * 128)

    return {
        "wt": np.ascontiguousarray(wt, np.float32),
        "gt2": np.ascontiguousarray(gt2, np.float32),
        "ftp": np.ascontiguousarray(ftp, np.float32),
        "bias": np.ascontiguousarray(b, np.float32).reshape(128, 1),
        "sel": _sel_const(),
    }


def run(feature, base, W, b, **spmd_kwargs):
    nc = _get_nc()
    in_maps = [_prep_core_inputs(feature, base, W, b, ci) for ci in range(NCORES)]
    res = run_bass_kernel_spmd(nc, in_maps, list(range(NCORES)), **spmd_kwargs)
    outs = []
    for ci in range(NCORES):
        o = res.results[ci]["out"].reshape(O, BLOC, D)
        outs.append(np.transpose(o, (1, 0, 2)))
    full = np.concatenate(outs, 0)
    return full, res


def kernel(feature, base, W, b):
    full, _ = run(feature, base, W, b)
    return full


# revision 44
# speedup vs baseline: 1.1994x; 1.0008x over previous
"""Trainium2 Bass kernel for nn_CINComp: out[b,o,d] = sum_{i,j} W[o,i*64+j]*feature[b,i,d]*base[b,j,d] + bias[o].

Sharding: data-parallel over batch B=1024 across 8 cores (128 batches/core).
Per-core algorithm (all shapes hardcoded):
  - contraction dim ij = 4096 split into 32 K-chunks of 128 = (2 i-rows x 64 j)
  - G is stored transposed+duplicated: gt2[p=(dup,j), (b,d)] so each K-chunk's
    G-factor is a static 128-partition tile slice
  - per chunk, a K=8 selector matmul on the PE broadcasts the chunk's 2 F-rows
    across the partition halves into PSUM (fp32r, 1 cyc/row)
  - DVE multiplies gt2-slice * F-broadcast -> P chunk (the outer-product block)
  - PE contracts W^T-chunk (fp32r) @ P accumulating out[o,(b,d)] in PSUM
  - ScalarE adds bias during PSUM->SBUF copy, DMA out
"""

import numpy as np

import concourse.bass as bass
import concourse.mybir as mybir
import concourse.tile as tile
from concourse.bass import ts
from concourse.bass_utils import run_bass_kernel_spmd

B, HK, H0, D, O = 1024, 64, 64, 32, 128
NCORES = 8
BLOC = B // NCORES          # 128 batches per core
GROUPS = 8                  # batch groups per core
GB = BLOC // GROUPS         # 16 batches per group
N = GB * D                  # 512 = matmul free dim per group
NCHUNK = 32                 # K chunks of 128 over ij=4096
F32 = mybir.dt.float32
F32R = mybir.dt.float32r

_CACHE = {}


def _sel_const() -> np.ndarray:
    # sel[k, c, m] = 1 iff k == 2c + (m // 64): K=128 selector per chunk c that
    # broadcasts F-row 2c+delta (at partition 2c+delta of the padded F tile)
    # to partition half delta. K=128 keeps the PE's fused weight load on the
    # fast path (FWL requires 128 weight rows).
    sel = np.zeros((128, NCHUNK, 128), np.float32)
    for c in range(NCHUNK):
        for m in range(128):
            sel[2 * c + (m // 64), c, m] = 1.0
    return sel.reshape(128, NCHUNK * 128)


def _strip_self_waits(nc: bass.Bass) -> None:
    """Transitively-minimal semaphore waits.

    Tile's add_semaphores pass is per-proc minimal but not transitively
    minimal, and several instruction structs (fused LDWEIGHTS+MATMUL, TS, TT,
    DMA descriptors) accept only ONE wait.  This pass computes completion
    vector clocks (sem id -> value) for every instruction, exploiting
    in-order FIFO execution (per engine; per DMA queue = per queue sem), and
    drops any wait already implied by the instruction's FIFO predecessor or
    by its remaining waits.
    """
    UPD = ("sem-inc", "sem-add-imm")
    insts = [i for bb in nc.m.functions[0].blocks for i in bb.instructions]

    # Sems driven by anything other than monotonic inc/add (e.g. the barrier
    # protocol's sem-sub) must keep their waits untouched.
    bad_sems = set()
    for i in insts:
        si = getattr(i, "sync_info", None)
        if si is None:
            continue
        for u in si.on_update:
            if u.sync_type != "semaphore" or u.update_mode not in UPD:
                bad_sems.add(u.id)

    def fifo_of(i):
        si = i.sync_info
        eng = str(getattr(i, "engine", None))
        if type(i).__name__ == "InstDMACopy" and si is not None:
            for u in si.on_update:
                if u.sync_type == "semaphore" and u.update_mode in UPD:
                    return ("q", u.id)
        return ("e", eng)

    # map sem threshold -> updater index, in program order (valid because all
    # updaters of one sem share a FIFO)
    cum: dict = {}
    event: dict = {}  # (sem, cum_value_after_update) -> inst index
    fifo_pred: dict = {}
    last_in_fifo: dict = {}
    metas = []
    for idx, i in enumerate(insts):
        si = getattr(i, "sync_info", None)
        f = fifo_of(i)
        fifo_pred[idx] = last_in_fifo.get(f)
        last_in_fifo[f] = idx
        ups = []
        if si is not None:
            for u in si.on_update:
                if u.sync_type == "semaphore" and u.update_mode in UPD:
                    cum[u.id] = cum.get(u.id, 0) + u.update_value
                    event[(u.id, cum[u.id])] = idx
                    ups.append((u.id, cum[u.id]))
        metas.append((si, ups))

    # resolve a wait (sem, k) to the earliest event index with cum >= k
    def resolve(sem, k):
        v = k
        while (sem, v) not in event:
            v += 1
            if v > cum.get(sem, 0):
                return None
        return event[(sem, v)]

    cvc: list = [None] * len(insts)

    def get_cvc(idx):
        if cvc[idx] is not None:
            return cvc[idx]
        # iterative DFS to avoid deep recursion
        stack = [idx]
        while stack:
            j = stack[-1]
            if cvc[j] is not None:
                stack.pop()
                continue
            si, ups = metas[j]
            deps = []
            p = fifo_pred[j]
            if p is not None:
                deps.append(p)
            if si is not None:
                for w in si.on_wait:
                    if (
                        w.sync_type == "semaphore"
                        and w.wait_mode == "sem-ge-imm"
                        and w.id not in bad_sems
                    ):
                        e = resolve(w.id, w.wait_value)
                        if e is not None and e != j:
                            deps.append(e)
            pending = [d for d in deps if cvc[d] is None]
            if pending:
                stack.extend(pending)
                continue
            stack.pop()
            vc: dict = {}
            for d in deps:
                for s, v in cvc[d].items():
                    if vc.get(s, 0) < v:
                        vc[s] = v
            if si is not None:
                for w in si.on_wait:
                    if (
                        w.sync_type == "semaphore"
                        and w.wait_mode == "sem-ge-imm"
                        and w.id not in bad_sems
                    ):
                        if vc.get(w.id, 0) < w.wait_value:
                            vc[w.id] = w.wait_value
            for s, v in ups:
                if vc.get(s, 0) < v:
                    vc[s] = v
            cvc[j] = vc
        return cvc[idx]

    for idx, i in enumerate(insts):
        si, _ups = metas[idx]
        if si is None or not si.on_wait:
            continue
        base: dict = {}
        p = fifo_pred[idx]
        if p is not None:
            base = dict(get_cvc(p))
        sem_waits = [
            w
            for w in si.on_wait
            if w.sync_type == "semaphore"
            and w.wait_mode == "sem-ge-imm"
            and w.id not in bad_sems
        ]
        other = [w for w in si.on_wait if w not in sem_waits]
        # strongest-first so one kept wait can subsume the rest
        def strength(w):
            e = resolve(w.id, w.wait_value)
            return len(get_cvc(e)) if e is not None else 0

        sem_waits.sort(key=strength, reverse=True)

        def wait_cvc(w):
            e = resolve(w.id, w.wait_value)
            vc = dict(get_cvc(e)) if e is not None else {}
            if vc.get(w.id, 0) < w.wait_value:
                vc[w.id] = w.wait_value
            return vc

        kept = sem_waits[:]
        changed = True
        while changed:
            changed = False
            for w in kept:
                cover = dict(base)
                for w2 in kept:
                    if w2 is w:
                        continue
                    for s, v in wait_cvc(w2).items():
                        if cover.get(s, 0) < v:
                            cover[s] = v
                if cover.get(w.id, 0) >= w.wait_value:
                    kept.remove(w)
                    changed = True
                    break
        if len(kept) + len(other) != len(si.on_wait):
            si.on_wait = other + kept


def _build_nc(strip: bool = True) -> bass.Bass:
    nc = bass.Bass()
    wt = nc.dram_tensor("wt", [128, NCHUNK * 128], F32R, kind="ExternalInput")
    gt2 = nc.dram_tensor("gt2", [128, BLOC * D], F32, kind="ExternalInput")
    ftp = nc.dram_tensor("ftp", [HK, BLOC * D], F32R, kind="ExternalInput")
    bias = nc.dram_tensor("bias", [128, 1], F32, kind="ExternalInput")
    out = nc.dram_tensor("out", [128, BLOC * D], F32, kind="ExternalOutput")
    sel_d = nc.dram_tensor("sel", [128, NCHUNK * 128], F32R, kind="ExternalInput")

    PAIRS = NCHUNK // 2  # chunks processed two-at-a-time

    with tile.TileContext(nc) as tc:
        with (
            tc.tile_pool(name="resident", bufs=1) as res,
            tc.tile_pool(name="p", bufs=3) as ppool,
            tc.tile_pool(name="osb", bufs=8) as opool,
            tc.tile_pool(name="tiny", bufs=2) as tpool,
            tc.tile_pool(name="fbc", bufs=3, space="PSUM") as fpool,
            tc.tile_pool(name="acc", bufs=2, space="PSUM") as apool,
        ):
            gt2_sb = res.tile([128, BLOC * D], F32)
            ft_sb = res.tile([128, BLOC * D], F32R)
            sel_sb = res.tile([128, NCHUNK * 128], F32R)
            wt_sb = res.tile([128, NCHUNK * 128], F32R)
            bias_sb = res.tile([128, 1], F32)

            # Quarter-sliced resident loads, interleaved so the first pairs'
            # operands land early and compute overlaps the remaining input
            # DMA. Each piece is "touched" with a 1-element self-copy on its
            # consumer engine: later readers then depend on that engine's sem
            # instead of the DMA queue (instruction structs accept only one
            # embedded wait), and the RAW dep keeps the ordering.
            Q = BLOC * D // 4
            nc.sync.dma_start(out=bias_sb[:], in_=bias[:])
            # rows 64-127 of ft are multiplied by zero selector weights; they
            # only need to be NaN-free, so memset them on idle GPSIMD instead
            # of shipping zeros over HBM.
            nc.gpsimd.memset(ft_sb[64:128, :].bitcast(F32), 0.0)
            nc.vector.tensor_copy(ft_sb[64:65, 0:1], ft_sb[64:65, 0:1])
            for q in range(4):
                nc.sync.dma_start(out=gt2_sb[:, ts(q, Q)], in_=gt2[:, ts(q, Q)])
                nc.sync.dma_start(out=ft_sb[0:HK, ts(q, Q)],
                                  in_=ftp[:, ts(q, Q)])
                nc.sync.dma_start(out=sel_sb[:, ts(q, Q)], in_=sel_d[:, ts(q, Q)])
                nc.sync.dma_start(out=wt_sb[:, ts(q, Q)], in_=wt[:, ts(q, Q)])
                for t in (ft_sb, sel_sb, gt2_sb, wt_sb):
                    nc.vector.tensor_copy(t[0:1, q * Q:q * Q + 1],
                                          t[0:1, q * Q:q * Q + 1])
            nc.vector.tensor_copy(bias_sb[0:1, 0:1], bias_sb[0:1, 0:1])
            tiny = tpool.tile([128, 1], F32, tag="tiny")
            nc.vector.tensor_copy(tiny[:], bias_sb[:, 0:1])

            # All pairs: DVE multiplies G in directly from the PSUM broadcast
            # tile (measured same DVE rate as SBUF-staged, and it frees ACT).
            LANES = ["B"] * PAIRS
            for g in range(GROUPS):
                acc = apool.tile([128, N], F32, tag="acc")
                for k in range(PAIRS):
                    lane = LANES[k]
                    c0, c1 = 2 * k, 2 * k + 1
                    # two K=128 broadcast matmuls -> one 2-bank PSUM tile
                    fbc = fpool.tile([128, 2 * N], F32, tag="fbc")
                    nc.tensor.matmul(fbc[:, 0:N], sel_sb[:, ts(c0, 128)],
                                     ft_sb[:, ts(g, N)], start=True, stop=True)
                    nc.tensor.matmul(fbc[:, N:2 * N], sel_sb[:, ts(c1, 128)],
                                     ft_sb[:, ts(g, N)], start=True, stop=True)
                    gview = gt2_sb[:, ts(g, N)][:, None, :].to_broadcast(
                        (128, 2, N))
                    p = ppool.tile([128, 2, N], F32R, tag="pd", bufs=4)
                    nc.vector.tensor_mul(
                        p[:], gview,
                        fbc[:].rearrange("a (b n) -> a b n", b=2))
                    nc.tensor.matmul(acc[:], wt_sb[:, ts(c0, 128)], p[:, 0, :],
                                     start=(k == 0), stop=False)
                    nc.tensor.matmul(acc[:], wt_sb[:, ts(c1, 128)], p[:, 1, :],
                                     start=False, stop=(k == PAIRS - 1))

                osb = opool.tile([128, N], F32, tag="osb")
                nc.vector.tensor_scalar(osb[:], acc[:], bias_sb[:, 0:1],
                                        None, mybir.AluOpType.add)
                nc.sync.dma_start(out=out[:, ts(g, N)], in_=osb[:])
                # WAR consumer: pulls the out-DMA's queue-sem onto the DVE
                # clock so the kernel-tail drain needs only one wait.
                nc.vector.tensor_copy(osb[0:1, 0:1], tiny[0:1, 0:1])
                prev_osb = osb

    if strip:
        _strip_self_waits(nc)
    return nc


def _get_nc() -> bass.Bass:
    if "nc" not in _CACHE:
        _CACHE["nc"] = _build_nc()
    return _CACHE["nc"]


def _prep_core_inputs(feature, base, W, b, ci):
    bsl = slice(ci * BLOC, (ci + 1) * BLOC)
    F = np.ascontiguousarray(feature[bsl], np.float32)  # (128, 64, 32)
    G = np.ascontiguousarray(base[bsl], np.float32)     # (128, 64, 32)

    Gt = np.transpose(G, (1, 0, 2))                      # (j, b, d)
    gt2 = np.concatenate([Gt, Gt], 0).reshape(128, BLOC * D)

    # ftp: F transposed to (i, b, d); device memsets the padding rows
    ftp = np.transpose(F, (1, 0, 2)).reshape(HK, BLOC * D)

    # wt[p, c, o] = W[o, 128c + p]
    wt = np.transpose(W.reshape(O, NCHUNK, 128), (2, 1, 0)).reshape(128, NCHUNK * 128)

    return {
        "wt": np.ascontiguousarray(wt, np.float32),
        "gt2": np.ascontiguousarray(gt2, np.float32),
        "ftp": np.ascontiguousarray(ftp, np.float32),
        "bias": np.ascontiguousarray(b, np.float32).reshape(128, 1),
        "sel": _sel_const(),
    }


def run(feature, base, W, b, **spmd_kwargs):
    nc = _get_nc()
    in_maps = [_prep_core_inputs(feature, base, W, b, ci) for ci in range(NCORES)]
    res = run_bass_kernel_spmd(nc, in_maps, list(range(NCORES)), **spmd_kwargs)
    outs = []
    for ci in range(NCORES):
        o = res.results[ci]["out"].reshape(O, BLOC, D)
        outs.append(np.transpose(o, (1, 0, 2)))
    full = np.concatenate(outs, 0)
    return full, res


def kernel(feature, base, W, b):
    full, _ = run(feature, base, W, b)
    return full


# revision 51
# speedup vs baseline: 1.2178x; 1.0153x over previous
"""Trainium2 Bass kernel for nn_CINComp: out[b,o,d] = sum_{i,j} W[o,i*64+j]*feature[b,i,d]*base[b,j,d] + bias[o].

Sharding: data-parallel over batch B=1024 across 8 cores (128 batches/core).
Per-core algorithm (all shapes hardcoded):
  - contraction dim ij = 4096 split into 32 K-chunks of 128 = (2 i-rows x 64 j)
  - G is stored transposed+duplicated: gt2[p=(dup,j), (b,d)] so each K-chunk's
    G-factor is a static 128-partition tile slice
  - per chunk, a K=8 selector matmul on the PE broadcasts the chunk's 2 F-rows
    across the partition halves into PSUM (fp32r, 1 cyc/row)
  - DVE multiplies gt2-slice * F-broadcast -> P chunk (the outer-product block)
  - PE contracts W^T-chunk (fp32r) @ P accumulating out[o,(b,d)] in PSUM
  - ScalarE adds bias during PSUM->SBUF copy, DMA out
"""

import numpy as np

import concourse.bass as bass
import concourse.mybir as mybir
import concourse.tile as tile
from concourse.bass import ts
from concourse.bass_utils import run_bass_kernel_spmd

B, HK, H0, D, O = 1024, 64, 64, 32, 128
NCORES = 8
BLOC = B // NCORES          # 128 batches per core
GROUPS = 8                  # batch groups per core
GB = BLOC // GROUPS         # 16 batches per group
N = GB * D                  # 512 = matmul free dim per group
NCHUNK = 32                 # K chunks of 128 over ij=4096
F32 = mybir.dt.float32
F32R = mybir.dt.float32r

_CACHE = {}


def _sel_const() -> np.ndarray:
    # sel[k, c, m] = 1 iff k == 2c + (m // 64): K=128 selector per chunk c that
    # broadcasts F-row 2c+delta (at partition 2c+delta of the padded F tile)
    # to partition half delta. K=128 keeps the PE's fused weight load on the
    # fast path (FWL requires 128 weight rows).
    sel = np.zeros((128, NCHUNK, 128), np.float32)
    for c in range(NCHUNK):
        for m in range(128):
            sel[2 * c + (m // 64), c, m] = 1.0
    return sel.reshape(128, NCHUNK * 128)


def _strip_self_waits(nc: bass.Bass) -> None:
    """Transitively-minimal semaphore waits.

    Tile's add_semaphores pass is per-proc minimal but not transitively
    minimal, and several instruction structs (fused LDWEIGHTS+MATMUL, TS, TT,
    DMA descriptors) accept only ONE wait.  This pass computes completion
    vector clocks (sem id -> value) for every instruction, exploiting
    in-order FIFO execution (per engine; per DMA queue = per queue sem), and
    drops any wait already implied by the instruction's FIFO predecessor or
    by its remaining waits.
    """
    UPD = ("sem-inc", "sem-add-imm")
    insts = [i for bb in nc.m.functions[0].blocks for i in bb.instructions]

    # Sems driven by anything other than monotonic inc/add (e.g. the barrier
    # protocol's sem-sub) must keep their waits untouched.
    bad_sems = set()
    for i in insts:
        si = getattr(i, "sync_info", None)
        if si is None:
            continue
        for u in si.on_update:
            if u.sync_type != "semaphore" or u.update_mode not in UPD:
                bad_sems.add(u.id)

    def fifo_of(i):
        si = i.sync_info
        eng = str(getattr(i, "engine", None))
        if type(i).__name__ == "InstDMACopy" and si is not None:
            for u in si.on_update:
                if u.sync_type == "semaphore" and u.update_mode in UPD:
                    return ("q", u.id)
        return ("e", eng)

    # map sem threshold -> updater index, in program order (valid because all
    # updaters of one sem share a FIFO)
    cum: dict = {}
    event: dict = {}  # (sem, cum_value_after_update) -> inst index
    fifo_pred: dict = {}
    last_in_fifo: dict = {}
    metas = []
    for idx, i in enumerate(insts):
        si = getattr(i, "sync_info", None)
        f = fifo_of(i)
        fifo_pred[idx] = last_in_fifo.get(f)
        last_in_fifo[f] = idx
        ups = []
        if si is not None:
            for u in si.on_update:
                if u.sync_type == "semaphore" and u.update_mode in UPD:
                    cum[u.id] = cum.get(u.id, 0) + u.update_value
                    event[(u.id, cum[u.id])] = idx
                    ups.append((u.id, cum[u.id]))
        metas.append((si, ups))

    # resolve a wait (sem, k) to the earliest event index with cum >= k
    def resolve(sem, k):
        v = k
        while (sem, v) not in event:
            v += 1
            if v > cum.get(sem, 0):
                return None
        return event[(sem, v)]

    cvc: list = [None] * len(insts)

    def get_cvc(idx):
        if cvc[idx] is not None:
            return cvc[idx]
        # iterative DFS to avoid deep recursion
        stack = [idx]
        while stack:
            j = stack[-1]
            if cvc[j] is not None:
                stack.pop()
                continue
            si, ups = metas[j]
            deps = []
            p = fifo_pred[j]
            if p is not None:
                deps.append(p)
            if si is not None:
                for w in si.on_wait:
                    if (
                        w.sync_type == "semaphore"
                        and w.wait_mode == "sem-ge-imm"
                        and w.id not in bad_sems
                    ):
                        e = resolve(w.id, w.wait_value)
                        if e is not None and e != j:
                            deps.append(e)
            pending = [d for d in deps if cvc[d] is None]
            if pending:
                stack.extend(pending)
                continue
            stack.pop()
            vc: dict = {}
            for d in deps:
                for s, v in cvc[d].items():
                    if vc.get(s, 0) < v:
                        vc[s] = v
            if si is not None:
                for w in si.on_wait:
                    if (
                        w.sync_type == "semaphore"
                        and w.wait_mode == "sem-ge-imm"
                        and w.id not in bad_sems
                    ):
                        if vc.get(w.id, 0) < w.wait_value:
                            vc[w.id] = w.wait_value
            for s, v in ups:
                if vc.get(s, 0) < v:
                    vc[s] = v
            cvc[j] = vc
        return cvc[idx]

    for idx, i in enumerate(insts):
        si, _ups = metas[idx]
        if si is None or not si.on_wait:
            continue
        base: dict = {}
        p = fifo_pred[idx]
        if p is not None:
            base = dict(get_cvc(p))
        sem_waits = [
            w
            for w in si.on_wait
            if w.sync_type == "semaphore"
            and w.wait_mode == "sem-ge-imm"
            and w.id not in bad_sems
        ]
        other = [w for w in si.on_wait if w not in sem_waits]
        # strongest-first so one kept wait can subsume the rest
        def strength(w):
            e = resolve(w.id, w.wait_value)
            return len(get_cvc(e)) if e is not None else 0

        sem_waits.sort(key=strength, reverse=True)

        def wait_cvc(w):
            e = resolve(w.id, w.wait_value)
            vc = dict(get_cvc(e)) if e is not None else {}
            if vc.get(w.id, 0) < w.wait_value:
                vc[w.id] = w.wait_value
            return vc

        kept = sem_waits[:]
        changed = True
        while changed:
            changed = False
            for w in kept:
                cover = dict(base)
                for w2 in kept:
                    if w2 is w:
                        continue
                    for s, v in wait_cvc(w2).items():
                        if cover.get(s, 0) < v:
                            cover[s] = v
                if cover.get(w.id, 0) >= w.wait_value:
                    kept.remove(w)
                    changed = True
                    break
        if len(kept) + len(other) != len(si.on_wait):
            si.on_wait = other + kept


def _build_nc(strip: bool = True) -> bass.Bass:
    nc = bass.Bass()
    wt = nc.dram_tensor("wt", [128, NCHUNK * 128], F32R, kind="ExternalInput")
    gt2 = nc.dram_tensor("gt2", [128, BLOC * D], F32, kind="ExternalInput")
    ftp = nc.dram_tensor("ftp", [HK, BLOC * D], F32R, kind="ExternalInput")
    bias = nc.dram_tensor("bias", [128, 1], F32, kind="ExternalInput")
    out = nc.dram_tensor("out", [128, BLOC * D], F32, kind="ExternalOutput")
    sel_d = nc.dram_tensor("sel", [128, NCHUNK * 128], F32R, kind="ExternalInput")

    PAIRS = NCHUNK // 2  # chunks processed two-at-a-time

    with tile.TileContext(nc) as tc:
        with (
            tc.tile_pool(name="resident", bufs=1) as res,
            tc.tile_pool(name="p", bufs=3) as ppool,
            tc.tile_pool(name="osb", bufs=8) as opool,
            tc.tile_pool(name="tiny", bufs=2) as tpool,
            tc.tile_pool(name="fbc", bufs=3, space="PSUM") as fpool,
            tc.tile_pool(name="acc", bufs=2, space="PSUM") as apool,
        ):
            gt2_sb = res.tile([128, BLOC * D], F32)
            ft_sb = res.tile([128, BLOC * D], F32R)
            sel_sb = res.tile([128, NCHUNK * 128], F32R)
            wt_sb = res.tile([128, NCHUNK * 128], F32R)
            bias_sb = res.tile([128, 1], F32)

            # Quarter-sliced resident loads, interleaved so the first pairs'
            # operands land early and compute overlaps the remaining input
            # DMA. Each piece is "touched" with a 1-element self-copy on its
            # consumer engine: later readers then depend on that engine's sem
            # instead of the DMA queue (instruction structs accept only one
            # embedded wait), and the RAW dep keeps the ordering.
            Q = BLOC * D // 4
            nc.sync.dma_start(out=bias_sb[:], in_=bias[:])
            # rows 64-127 of ft are multiplied by zero selector weights; they
            # only need to be NaN-free, so memset them on idle GPSIMD instead
            # of shipping zeros over HBM.
            nc.gpsimd.memset(ft_sb[64:128, :].bitcast(F32), 0.0)
            nc.vector.tensor_copy(ft_sb[64:65, 0:1], ft_sb[64:65, 0:1])
            for q in range(4):
                nc.sync.dma_start(out=gt2_sb[:, ts(q, Q)], in_=gt2[:, ts(q, Q)])
                nc.sync.dma_start(out=ft_sb[0:HK, ts(q, Q)],
                                  in_=ftp[:, ts(q, Q)])
                nc.sync.dma_start(out=sel_sb[:, ts(q, Q)], in_=sel_d[:, ts(q, Q)])
                nc.sync.dma_start(out=wt_sb[:, ts(q, Q)], in_=wt[:, ts(q, Q)])
                for t in (ft_sb, sel_sb, gt2_sb, wt_sb):
                    nc.vector.tensor_copy(t[0:1, q * Q:q * Q + 1],
                                          t[0:1, q * Q:q * Q + 1])
            nc.vector.tensor_copy(bias_sb[0:1, 0:1], bias_sb[0:1, 0:1])
            tiny = tpool.tile([128, 1], F32, tag="tiny")
            nc.vector.tensor_copy(tiny[:], bias_sb[:, 0:1])

            # All pairs: DVE multiplies G in directly from the PSUM broadcast
            # tile (measured same DVE rate as SBUF-staged, and it frees ACT).
            LANES = ["B"] * PAIRS
            for g in range(GROUPS):
                acc = apool.tile([128, N], F32, tag="acc")
                for k in range(PAIRS):
                    lane = LANES[k]
                    c0, c1 = 2 * k, 2 * k + 1
                    # two K=128 broadcast matmuls -> one 2-bank PSUM tile
                    fbc = fpool.tile([128, 2 * N], F32, tag="fbc")
                    nc.tensor.matmul(fbc[:, 0:N], sel_sb[:, ts(c0, 128)],
                                     ft_sb[:, ts(g, N)], start=True, stop=True)
                    nc.tensor.matmul(fbc[:, N:2 * N], sel_sb[:, ts(c1, 128)],
                                     ft_sb[:, ts(g, N)], start=True, stop=True)
                    gview = gt2_sb[:, ts(g, N)][:, None, :].to_broadcast(
                        (128, 2, N))
                    p = ppool.tile([128, 2, N], F32R, tag="pd", bufs=4)
                    nc.vector.tensor_mul(
                        p[:], gview,
                        fbc[:].rearrange("a (b n) -> a b n", b=2))
                    nc.tensor.matmul(acc[:], wt_sb[:, ts(c0, 128)], p[:, 0, :],
                                     start=(k == 0), stop=False)
                    nc.tensor.matmul(acc[:], wt_sb[:, ts(c1, 128)], p[:, 1, :],
                                     start=False, stop=(k == PAIRS - 1))

                osb = opool.tile([128, N], F32, tag="osb")
                nc.vector.tensor_scalar(osb[:], acc[:], bias_sb[:, 0:1],
                                        None, mybir.AluOpType.add)
                nc.sync.dma_start(out=out[:, ts(g, N)], in_=osb[:])
                # WAR consumer: pulls the out-DMA's queue-sem onto the DVE
                # clock so the kernel-tail drain needs only one wait.
                nc.vector.tensor_copy(osb[0:1, 0:1], tiny[0:1, 0:1])

    if strip:
        _strip_self_waits(nc)
    return nc


def _get_nc() -> bass.Bass:
    if "nc" not in _CACHE:
        _CACHE["nc"] = _build_nc()
    return _CACHE["nc"]


def _prep_core_inputs(feature, base, W, b, ci):
    bsl = slice(ci * BLOC, (ci + 1) * BLOC)
    F = np.ascontiguousarray(feature[bsl], np.float32)  # (128, 64, 32)
    G = np.ascontiguousarray(base[bsl], np.float32)     # (128, 64, 32)

    Gt = np.transpose(G, (1, 0, 2))                      # (j, b, d)
    gt2 = np.concatenate([Gt, Gt], 0).reshape(128, BLOC * D)

    # ftp: F transposed to (i, b, d); device memsets the padding rows
    ftp = np.transpose(F, (1, 0, 2)).reshape(HK, BLOC * D)

    # wt[p, c, o] = W[o, 128c + p]
    wt = np.transpose(W.reshape(O, NCHUNK, 128), (2, 1, 0)).reshape(128, NCHUNK * 128)

    return {
        "wt": np.ascontiguousarray(wt, np.float32),
        "gt2": np.ascontiguousarray(gt2, np.float32),
        "ftp": np.ascontiguousarray(ftp, np.float32),
        "bias": np.ascontiguousarray(b, np.float32).reshape(128, 1),
        "sel": _sel_const(),
    }


def run(feature, base, W, b, **spmd_kwargs):
    nc = _get_nc()
    in_maps = [_prep_core_inputs(feature, base, W, b, ci) for ci in range(NCORES)]
    res = run_bass_kernel_spmd(nc, in_maps, list(range(NCORES)), **spmd_kwargs)
    outs = []
    for ci in range(NCORES):
        o = res.results[ci]["out"].reshape(O, BLOC, D)
        outs.append(np.transpose(o, (1, 0, 2)))
    full = np.concatenate(outs, 0)
    return full, res


def kernel(feature, base, W, b):
    full, _ = run(feature, base, W, b)
    return full
